# revision 11
# baseline (speedup 1.0000x reference)
"""Trainium2 Bass kernel for CostVolumePrompt (masked-softmax cost volume).

Math per (b, h) pair (W=512, C=128):
  vol[i, j] = dot(lfeat[:, i], rfeat[:, j]) / sqrt(C)
  prob      = softmax(vol, axis=j) * tril          (mask applied AFTER softmax)
  corresp_i = sum_j prob[i,j]*j ;  conf_i = max_j prob[i,j]
  disp_i    = max((i - corresp_i)/W, 0.1)
  out       = [fx*baseline/lfar / disp, conf]

TRANSPOSED-LAYOUT design: volT[j, i] = rfeat_chunk^T @ lfeat with j on
partitions (4 j-tiles of 128).  The per-row-i reductions over j then become
PE matmuls (contraction over partitions):
  red[2, 512] psum, rows [den | s1]:
    - "loraw" per jt: lhsT=[ones|0]   over eT[jt][:, 0:128(jt+1)]   (den)
    - "suffix" per jt: lhsT=[ones|w_jt] over eT[jt][:, 128(jt+1):512]
      (den + s1 on the strictly-lower region, w_jt[p] = 128jt+p)
    - "dm" per jt: lhsT=[0|w_jt] over the DVE-premasked diagonal block
      (s1 on the diagonal)
conf-max: DVE assembles mx[p, i] = max over tiles of masked eT (diag blocks
via 2x TT-mult with a tril mask, suffix via in-place TT-max chain), PE
transposes mx, DVE reduce_max gives maxE[128, 4] per pair directly in the
finals layout.  den/s1 rows are cast psum->sbuf by DVE and scattered into
the [128, 256] finals accumulators by sbuf->sbuf DMA.

Per-core work: H/8 = 16 rows x 4 batches = 64 pairs.
"""

import math
import numpy as np
from contextlib import ExitStack

import concourse.bass as bass
import concourse.bacc as bacc
import concourse.tile as tile
from concourse import mybir
from concourse._compat import with_exitstack
from concourse.bass_utils import run_bass_kernel_spmd

B, V, C, H, W = 4, 2, 128, 128, 512
NCORES = 8
HLOC = H // NCORES          # 16 h-rows per core
NT = HLOC * 4               # 64 finals columns per batch
NC = B * NT                 # 256 finals columns total (col = b*64 + h*4 + it)
SCALE = 1.0 / math.sqrt(C)
MIN_DISP = 0.1

F32 = mybir.dt.float32
F16 = mybir.dt.float16
ALU = mybir.AluOpType


@with_exitstack
def _body(ctx: ExitStack, tc: "tile.TileContext", io: dict):
    nc = tc.nc
    lfeat, rfeat = io["lfeat"], io["rfeat"]

    singles = ctx.enter_context(tc.tile_pool(name="singles", bufs=1))
    feats = ctx.enter_context(tc.tile_pool(name="feats", bufs=6))
    epool = ctx.enter_context(tc.tile_pool(name="epool", bufs=3))
    mxpool = ctx.enter_context(tc.tile_pool(name="mxpool", bufs=3))
    stpool = ctx.enter_context(tc.tile_pool(name="stpool", bufs=3))
    psV = ctx.enter_context(tc.tile_pool(name="psV", bufs=2, space="PSUM"))
    psR = ctx.enter_context(tc.tile_pool(name="psR", bufs=2, space="PSUM"))
    psT = ctx.enter_context(tc.tile_pool(name="psT", bufs=2, space="PSUM"))

    # ---- constants ----
    maskT = singles.tile([128, 512], F16, tag="maskT")     # 4x tril-T blocks
    nc.sync.dma_start(out=maskT[:], in_=io["maskT"][:, :])
    w4 = singles.tile([128, 16], F16, tag="w4")            # per jt: [1,w,0,w]
    nc.sync.dma_start(out=w4[:], in_=io["w4"][:, :])
    loT = singles.tile([128, 2], F16, tag="loT")           # [1, 0]
    nc.sync.dma_start(out=loT[:], in_=io["loT"][:, :])
    ident = singles.tile([128, 128], F16, tag="ident")
    nc.sync.dma_start(out=ident[:], in_=io["ident"][:, :])
    ivec = singles.tile([128, NC], F32, tag="ivec")
    nc.sync.dma_start(out=ivec[:], in_=io["ivec"][:, :])
    scol = singles.tile([128, NC], F32, tag="scol")
    nc.sync.dma_start(out=scol[:], in_=io["scol"][:, :])

    # ---- finals accumulators (persist across pairs) ----
    dnc = singles.tile([128, NC], F32, tag="dnc")   # denom
    s1c = singles.tile([128, NC], F32, tag="s1c")   # s1
    c1c = singles.tile([128, NC], F32, tag="c1c")   # maxE

    def emit_front(b, h):
        """S0: DMA in, main matmuls, exp."""
        lf = feats.tile([128, W], F16, tag="lf")
        rf = feats.tile([128, W], F16, tag="rf")
        nc.sync.dma_start(out=lf[:], in_=lfeat[b, :, h, :])
        nc.sync.dma_start(out=rf[:], in_=rfeat[b, :, h, :])
        eT = epool.tile([128, 4 * W], F16, tag="eT")
        for half in range(2):
            vh = psV.tile([128, 2 * W], F32, tag="vh")
            for q in range(2):
                jt = 2 * half + q
                nc.tensor.matmul(vh[:, q * W:(q + 1) * W],
                                 rf[:, jt * 128:(jt + 1) * 128], lf[:],
                                 start=True, stop=True)
            nc.scalar.activation(out=eT[:, half * 2 * W:(half + 1) * 2 * W],
                                 in_=vh[:],
                                 func=mybir.ActivationFunctionType.Exp,
                                 scale=SCALE)
        # diag premask on GPS -> mx blocks (GPS is slow: give it a full
        # pipeline stage of slack before the dm matmuls consume mx)
        mx = mxpool.tile([128, 512], F16, tag="mx")
        for jt in range(4):
            nc.gpsimd.tensor_tensor(
                out=mx[:, jt * 128:(jt + 1) * 128],
                in0=eT[:, jt * W + jt * 128: jt * W + (jt + 1) * 128],
                in1=maskT[:, jt * 128:(jt + 1) * 128],
                op=ALU.mult)
        return eT, mx

    def emit_mid(eT, mx, col0):
        """S1: reduction matmuls, max chain, den/s1 cast+scatter."""
        red = psR.tile([2, 512], F32, tag="red")
        # den: full-coverage raw matmuls (loraw jt=3 first: start=True full width)
        nc.tensor.matmul(red[:, 0:512], loT[:], eT[:, 3 * W:3 * W + 512],
                         start=True, stop=True)
        for jt in range(3):
            c1 = 128 * (jt + 1)
            nc.tensor.matmul(red[:, 0:c1], loT[:], eT[:, jt * W: jt * W + c1],
                             start=False, stop=True, skip_group_check=True)
        # suffix (strictly-lower): den+s1
        for jt in range(3):
            c0 = 128 * (jt + 1)
            nc.tensor.matmul(red[:, c0:512], w4[:, 4 * jt:4 * jt + 2],
                             eT[:, jt * W + c0: jt * W + 512],
                             start=False, stop=True, skip_group_check=True)
        # diag-masked s1 contributions
        for jt in range(4):
            nc.tensor.matmul(red[:, 128 * jt:128 * (jt + 1)],
                             w4[:, 4 * jt + 2:4 * jt + 4],
                             mx[:, 128 * jt:128 * (jt + 1)],
                             start=False, stop=True, skip_group_check=True)
        # max chain: mx[:, c] |= max over eT[jt][:, c] for c >= 128*(jt+1)
        for jt in range(3):
            c0 = 128 * (jt + 1)
            nc.vector.tensor_tensor(out=mx[:, c0:512], in0=mx[:, c0:512],
                                    in1=eT[:, jt * W + c0: jt * W + 512],
                                    op=ALU.max)
        # cast den/s1 rows to sbuf f32 into the per-batch staging buffer
        h = (col0 % NT) // 4
        if h == 0:
            emit_mid.stg = stpool.tile([2, HLOC * 512], F32, tag="stg")
        stg = emit_mid.stg
        nc.vector.tensor_copy(out=stg[:, h * 512:(h + 1) * 512], in_=red[:])
        if h == HLOC - 1:
            # roundtrip via DRAM scratch to transpose the whole batch into
            # the partition-major finals accumulators
            b = col0 // NT
            nc.sync.dma_start(out=io["scr"][b, :, :, :],
                              in_=stg[:].rearrange("r (h x) -> r h x", h=HLOC))
            nc.sync.dma_start(
                out=dnc[:, b * NT:(b + 1) * NT],
                in_=io["scr"][b, 0, :, :].rearrange("h (t x) -> x h t", t=4))
            nc.sync.dma_start(
                out=s1c[:, b * NT:(b + 1) * NT],
                in_=io["scr"][b, 1, :, :].rearrange("h (t x) -> x h t", t=4))
        return

    def emit_back(mx, col0):
        """S2: transpose mx, reduce max."""
        mxT = psT.tile([128, 512], F16, tag="mxT")
        for it in range(4):
            nc.tensor.transpose(mxT[:, it * 128:(it + 1) * 128],
                                mx[:, it * 128:(it + 1) * 128], ident[:])
        nc.vector.tensor_reduce(
            out=c1c[:, col0:col0 + 4],
            in_=mxT[:].rearrange("p (t x) -> p t x", t=4),
            axis=mybir.AxisListType.X, op=ALU.max)

    pairs = [(b, h) for b in range(B) for h in range(HLOC)]
    state = []  # entries: (eT, mx, col0); mid done flags via position
    for k, (b, h) in enumerate(pairs):
        eT, mx = emit_front(b, h)
        col0 = b * NT + h * 4
        state.append((eT, mx, col0))
        if len(state) >= 2:
            eT1, mx1, col1 = state[-2]
            emit_mid(eT1, mx1, col1)
        if len(state) >= 3:
            _, mx2, col2 = state[-3]
            emit_back(mx2, col2)
            state.pop(0)
    eTl, mxl, coll = state[-1]
    emit_mid(eTl, mxl, coll)
    _, mx2, col2 = state[-2]
    emit_back(mx2, col2)
    emit_back(mxl, coll)

    # ---- batched finals over all 256 columns ----
    fin = ctx.enter_context(tc.tile_pool(name="fin", bufs=1))
    r = fin.tile([128, NC], F32, tag="r")
    nc.vector.reciprocal_approx_fast(out=r[:], in_=dnc[:])
    cf = fin.tile([128, NC], F32, tag="cf")
    nc.vector.tensor_tensor(out=cf[:], in0=c1c[:], in1=r[:], op=ALU.mult)
    cor = fin.tile([128, NC], F32, tag="cor")
    nc.vector.tensor_tensor(out=cor[:], in0=s1c[:], in1=r[:], op=ALU.mult)
    dd = fin.tile([128, NC], F32, tag="dd")
    nc.vector.tensor_tensor(out=dd[:], in0=ivec[:], in1=cor[:], op=ALU.subtract)
    dcl = fin.tile([128, NC], F32, tag="dcl")
    nc.vector.tensor_scalar(
        out=dcl[:], in0=dd[:], scalar1=1.0 / W, scalar2=MIN_DISP,
        op0=ALU.mult, op1=ALU.max)
    r2 = fin.tile([128, NC], F32, tag="r2")
    nc.vector.reciprocal_approx_fast(out=r2[:], in_=dcl[:])
    od = fin.tile([128, NC], F32, tag="od")
    nc.vector.tensor_tensor(out=od[:], in0=r2[:], in1=scol[:], op=ALU.mult)
    nc.sync.dma_start(out=io["out_dc"][:, :], in_=od[:])
    nc.sync.dma_start(out=io["out_cf"][:, :], in_=cf[:])


_NC_CACHE = None


def _build_nc():
    global _NC_CACHE
    if _NC_CACHE is not None:
        return _NC_CACHE
    nc = bacc.Bacc("TRN2", target_bir_lowering=False, debug=False)
    io = {
        "lfeat": nc.dram_tensor("lfeat", (B, C, HLOC, W), F16, kind="ExternalInput"),
        "rfeat": nc.dram_tensor("rfeat", (B, C, HLOC, W), F16, kind="ExternalInput"),
        "maskT": nc.dram_tensor("maskT", (128, 512), F16, kind="ExternalInput"),
        "w4": nc.dram_tensor("w4", (128, 16), F16, kind="ExternalInput"),
        "loT": nc.dram_tensor("loT", (128, 2), F16, kind="ExternalInput"),
        "ident": nc.dram_tensor("ident", (128, 128), F16, kind="ExternalInput"),
        "ivec": nc.dram_tensor("ivec", (128, NC), F32, kind="ExternalInput"),
        "scol": nc.dram_tensor("scol", (128, NC), F32, kind="ExternalInput"),
        "out_dc": nc.dram_tensor("out_dc", (128, NC), F32, kind="ExternalOutput"),
        "out_cf": nc.dram_tensor("out_cf", (128, NC), F32, kind="ExternalOutput"),
        "scr": nc.dram_tensor("scr", (B, 2, HLOC, 512), F32, kind="Internal"),
    }
    with tile.TileContext(nc) as tc:
        _body(tc, io)
    nc.compile()
    _NC_CACHE = nc
    return nc


def _host_constants(scales):
    p = np.arange(128)
    # maskT[p, 128*jt + c] = 1 if p <= c  (j = 128jt+p <= i = 128jt+c)
    blk = (p[:, None] <= np.arange(128)[None, :]).astype(np.float16)
    maskT = np.tile(blk, (1, 4))
    # w4 per jt: [ones, p+128jt, 0, p+128jt]
    w4 = np.zeros((128, 16), np.float16)
    for jt in range(4):
        w4[:, 4 * jt] = 1.0
        w4[:, 4 * jt + 1] = (p + 128 * jt).astype(np.float16)
        w4[:, 4 * jt + 3] = (p + 128 * jt).astype(np.float16)
    loT = np.zeros((128, 2), np.float16)
    loT[:, 0] = 1.0
    ident = np.eye(128, dtype=np.float16)
    # finals col c = b*64 + h*4 + it ; row index i = 128*it + p
    itcol = np.tile(np.arange(4), B * HLOC)
    ivec = ((128.0 * itcol)[None, :] + p[:, None]).astype(np.float32)
    bcol = np.repeat(np.arange(B), NT)
    scol = np.broadcast_to(scales[bcol][None, :], (128, NC)).astype(np.float32)
    return maskT, w4, loT, ident, ivec, scol


def kernel(feat, extri, intri, near, far, _run_kwargs=None, _core_ids=None):
    feat = np.asarray(feat, dtype=np.float32)
    extri = np.asarray(extri, dtype=np.float32)
    intri = np.asarray(intri, dtype=np.float32)
    far = np.asarray(far, dtype=np.float32)

    fx = intri[:, 0, 0, 0]
    baseline = np.linalg.norm(extri[:, 0, :3, 3] - extri[:, 1, :3, 3], axis=-1)
    lfar = far[:, 0]
    scales = (fx * baseline / lfar).astype(np.float32)

    maskT, w4, loT, ident, ivec, scol = _host_constants(scales)
    core_ids = list(range(NCORES)) if _core_ids is None else _core_ids

    feat_f16 = feat.astype(np.float16)

    in_maps = []
    for ci in range(len(core_ids)):
        hs = slice(ci * HLOC, (ci + 1) * HLOC)
        in_maps.append({
            "lfeat": np.ascontiguousarray(feat_f16[:, 0, :, hs, :]),
            "rfeat": np.ascontiguousarray(feat_f16[:, 1, :, hs, :]),
            "maskT": maskT, "w4": w4, "loT": loT, "ident": ident,
            "ivec": ivec, "scol": scol,
        })

    nc = _build_nc()
    res = run_bass_kernel_spmd(nc, in_maps, core_ids=core_ids,
                               **(_run_kwargs or {}))

    out = np.zeros((B, 1, 2, H, W), dtype=np.float32)
    for ci in range(len(core_ids)):
        h0 = ci * HLOC
        dc = res.results[ci]["out_dc"]          # (128, 256) col = b*64+h*4+it
        cfv = res.results[ci]["out_cf"]
        # row i = 128*it + p ; out[b, h, i]
        dc = dc.reshape(128, B, HLOC, 4).transpose(1, 2, 3, 0).reshape(B, HLOC, W)
        cfv = cfv.reshape(128, B, HLOC, 4).transpose(1, 2, 3, 0).reshape(B, HLOC, W)
        out[:, 0, 0, h0:h0 + HLOC, :] = dc
        out[:, 0, 1, h0:h0 + HLOC, :] = cfv
    if _run_kwargs:
        kernel.last_results = res
    return out
